# revision 32
# baseline (speedup 1.0000x reference)
"""Trainium2 Bass kernel for LuluAttention (gated GQA attention + RoPE).

Sharding over 8 NeuronCores: core = b*4 + g where b = batch (2), g = head
group (4). Each core computes 4 Q heads + their shared KV head for one batch
element, plus the matching gate slice, and a partial o_proj output
(contraction over its 512 attn dims). Host sums the 4 partials per batch.

Two-pass structure per core:
  Pass A (chunks 0..3): x chunk load -> q/k projections + RoPE -> gate
    (sigmoid) -> v projection. All activations persist in SBUF in transposed
    layout ([dim, seq]) so attention needs no on-chip transposes.
  Pass B (chunks 0..3): causal attention (scoresT = kT.T @ qT per k-tile,
    exp batched 2 tiles per ACTIVATE, triangular-block mask on the diagonal
    128-col block only, attnT accumulated in PSUM), denominator via a dense
    ones-matmul chain over retained prob tiles, reciprocal_approx_fast,
    broadcast via K=1 matmul, gate+normalize muls, then partial o_proj.

This keeps the exp table set (pass B) and sigmoid set (pass A) from
thrashing, keeps TensorE dense (no long PE-idle gaps -> HAM stays at 8/8),
and slices diagonal-tile matmuls to skip the causally-masked column ranges.
"""

import numpy as np
import ml_dtypes
from contextlib import ExitStack

import concourse.bass as bass
import concourse.bacc as bacc
import concourse.tile as tile
from concourse import mybir
from concourse.bass_utils import run_bass_kernel_spmd

BF16 = ml_dtypes.bfloat16

HIDDEN = 2048
B = 2
S_FULL = 2048
P = 128
CH = 512               # seq chunk width
QH = 4                 # q heads per core
DQ = QH * P            # 512 q dims per core
KT = HIDDEN // P       # 16 contraction tiles
SCALE = 1.0 / float(np.sqrt(128.0))
ROPE_THETA = 10000.0


def build_program(S=S_FULL):
    f32 = mybir.dt.float32
    bf16 = mybir.dt.bfloat16
    tanh = mybir.ActivationFunctionType.Tanh
    expf = mybir.ActivationFunctionType.Exp

    NCH = S // CH
    ST = CH // P           # 4 seq sub-tiles per chunk

    nc = bacc.Bacc("TRN2", debug=False, target_bir_lowering=False)

    xT = nc.declare_dram_parameter("xT", [HIDDEN, S], bf16, False)
    wq = nc.declare_dram_parameter("wq", [HIDDEN, DQ], bf16, False)
    wkv = nc.declare_dram_parameter("wkv", [HIDDEN, 2 * P], bf16, False)
    wg = nc.declare_dram_parameter("wg", [HIDDEN, DQ], bf16, False)
    wo = nc.declare_dram_parameter("wo", [DQ, HIDDEN], bf16, False)
    bg = nc.declare_dram_parameter("bg", [DQ], f32, False)
    cosT = nc.declare_dram_parameter("cosT", [P, S], bf16, False)
    sinT = nc.declare_dram_parameter("sinT", [P, S], bf16, False)
    msk = nc.declare_dram_parameter("msk", [P, P], bf16, False)
    out = nc.declare_dram_parameter("out", [S, HIDDEN], bf16, True)

    with tile.TileContext(nc) as tc, ExitStack() as ctx:
        wpool = ctx.enter_context(tc.tile_pool(name="weights", bufs=1))
        qkv = ctx.enter_context(tc.tile_pool(name="qkv", bufs=1))

        # ---- persistent tiles; DMAs are issued in ramp-critical order ----
        # (sync-ring DMAs drain FIFO, so the first q-projection's operands
        # must be first in line: wq block 0, then the first x chunk.)
        # wq loaded in 4 contraction-row splits (1KB HBM rows, and the first
        # q chain can start as soon as split 0 lands via subtile deps)
        wq_sb = wpool.tile([P, KT, DQ], bf16, tag="wq")
        wkv_sb = wpool.tile([P, KT, 2 * P], bf16, tag="wkv")
        wk_sb = wkv_sb[:, :, 0:P]
        wv_sb = wkv_sb[:, :, P:2 * P]
        cos_sb = wpool.tile([P, S], bf16, tag="cos")
        sin_sb = wpool.tile([P, S], bf16, tag="sin")
        wg_sb = wpool.tile([P, KT, DQ], bf16, tag="wg")
        bg_sb = wpool.tile([P, QH], f32, tag="bg")
        msk_sb = wpool.tile([P, P], bf16, tag="msk")
        wo_sb = wpool.tile([P, QH, HIDDEN], bf16, tag="wo")

        def dma_wq_split(h):
            nc.sync.dma_start(
                out=wq_sb[:, h * KT // 4:(h + 1) * KT // 4, :],
                in_=wq[h * HIDDEN // 4:(h + 1) * HIDDEN // 4, :].rearrange(
                    "(kt p) n -> p kt n", p=P),
            )
        dma_wq_split(0)
        # denominator matmul: all-2.0 stationary operand broadcasts
        # 2*sum(exp) to every PSUM partition, and the 2x absorbs the gate's
        # (1 + tanh)/2 affine: ag = at*(1+tanh) * 1/(2*sum(exp))
        twos = wpool.tile([P, P], bf16, tag="twos")
        nc.vector.memset(twos, 2.0)

        # preload the exp/tanh ACT table set during the DMA ramp so no
        # table switch lands mid-kernel
        warm = wpool.tile([1, 2], f32, tag="warm")
        nc.vector.memset(warm, 1.0)
        nc.scalar.activation(out=warm[:, 1:2], in_=warm[:, 0:1], func=tanh)
        nc.scalar.activation(out=warm[:, 0:1], in_=warm[:, 1:2], func=expf)

        def load_weights_early():
            nc.sync.dma_start(out=cos_sb, in_=cosT[:, :])
            nc.sync.dma_start(out=sin_sb, in_=sinT[:, :])
            nc.sync.dma_start(
                out=wkv_sb, in_=wkv[:, :].rearrange("(kt p) n -> p kt n", p=P))
            nc.sync.dma_start(
                out=wg_sb, in_=wg[:, :].rearrange("(kt p) n -> p kt n", p=P))
            nc.sync.dma_start(out=bg_sb, in_=bg[:].rearrange("(h p) -> p h", p=P))
            nc.sync.dma_start(out=msk_sb, in_=msk[:, :])

        def load_weights_late():
            nc.sync.dma_start(
                out=wo_sb, in_=wo[:, :].rearrange("(dt p) n -> p dt n", p=P))

        # persistent per-core activations (transposed layouts)
        qro = qkv.tile([P, QH, S], bf16, tag="qro")
        kro = qkv.tile([P, S], bf16, tag="kro")
        v_sb = qkv.tile([P, S // P, P], bf16, tag="v")
        gt = qkv.tile([P, QH, S], bf16, tag="gt")

        # ================= PASS A: projections =================
        with tc.tile_pool(name="passA", bufs=2) as xpool, \
             tc.tile_pool(name="workA", bufs=4) as work, \
             tc.tile_pool(name="psA", bufs=4, space="PSUM") as psA:
            def load_xc(c):
                # split so the first q chain can start on the first piece;
                # for chunk 0, interleave with the wq row-splits so both
                # operands stream together
                xc = xpool.tile([P, KT, CH], bf16, tag="xc", name=f"xc{c}")
                ccs = slice(c * CH, (c + 1) * CH)
                nsp = 4 if c == 0 else 2
                for h in range(nsp):
                    kt0, kt1 = h * KT // nsp, (h + 1) * KT // nsp
                    nc.sync.dma_start(
                        out=xc[:, kt0:kt1, :],
                        in_=xT[kt0 * P:kt1 * P, ccs].rearrange(
                            "(kt p) n -> p kt n", p=P),
                    )
                    if c == 0 and h < 3:
                        dma_wq_split(h + 1)
                return xc

            xc_next = load_xc(0)
            load_weights_early()
            for c in range(NCH):
                cs = slice(c * CH, (c + 1) * CH)
                xc = xc_next
                if c + 1 < NCH:
                    xc_next = load_xc(c + 1)
                if c == 2:
                    load_weights_late()

                # q heads + k + v (transposed); RoPE applied to q/k out of PSUM
                for qh in range(QH + 2):
                    ps = psA.tile([P, CH], f32, tag="proj")
                    for kt in range(KT):
                        lhs = (
                            wq_sb[:, kt, qh * P:(qh + 1) * P]
                            if qh < QH
                            else (wk_sb if qh == QH else wv_sb)[:, kt, :]
                        )
                        nc.tensor.matmul(
                            ps, lhs, xc[:, kt, :], start=(kt == 0), stop=(kt == KT - 1)
                        )
                    if qh == QH + 1:
                        # v: cast out of PSUM, then xbar-transpose to [s, d]
                        # (scalar HWDGE ring: keeps it off the big-load ring)
                        vt = work.tile([P, CH], bf16, tag="vt")
                        nc.scalar.copy(out=vt, in_=ps)
                        for st in range(ST):
                            nc.scalar.dma_start_transpose(
                                out=v_sb[:, c * ST + st, :],
                                in_=vt[:, st * P:(st + 1) * P],
                            )
                        continue
                    qf = work.tile([P, CH], bf16, tag="qf")
                    nc.scalar.copy(out=qf, in_=ps)
                    # rotate-half via the scalar HWDGE ring: tiny and
                    # latency-critical, must not queue behind weight loads
                    rot = work.tile([P, CH], bf16, tag="rot")
                    nc.scalar.dma_start(out=rot[0:64, :], in_=qf[64:128, :])
                    nc.scalar.dma_start(out=rot[64:128, :], in_=qf[0:64, :])
                    t1 = work.tile([P, CH], bf16, tag="t1")
                    nc.vector.tensor_mul(t1, qf, cos_sb[:, cs])
                    t2 = work.tile([P, CH], bf16, tag="t2")
                    nc.vector.tensor_mul(t2, rot, sin_sb[:, cs])
                    dst = qro[:, qh, cs] if qh < QH else kro[:, cs]
                    nc.vector.tensor_add(dst, t1, t2)

                # gate heads: tanh((x @ Wg + bg)/2), transposed layout.
                # sigmoid = (1+tanh)/2 is finished in pass B's normalize
                # multiply (tanh shares the exp ACT table set; sigmoid won't).
                for qh in range(QH):
                    ps = psA.tile([P, CH], f32, tag="proj")
                    for kt in range(KT):
                        nc.tensor.matmul(
                            ps,
                            wg_sb[:, kt, qh * P:(qh + 1) * P],
                            xc[:, kt, :],
                            start=(kt == 0),
                            stop=(kt == KT - 1),
                        )
                    nc.scalar.activation(
                        out=gt[:, qh, cs],
                        in_=ps,
                        func=tanh,
                        bias=bg_sb[:, qh:qh + 1],
                        scale=0.5,
                    )

        # ================= PASS B: attention + o_proj =================
        # PSUM budget (8 banks): sc tag [P,2,CH] x2 bufs = 4 banks (shared by
        # attention score pairs and o_proj output pairs), attn tag x2 = 2,
        # sm tag x2 = 2 (denominator and broadcast cycle the same slots).
        with tc.tile_pool(name="prp", bufs=2) as prp, \
             tc.tile_pool(name="agp", bufs=2) as agp, \
             tc.tile_pool(name="workB", bufs=2) as workB, \
             tc.tile_pool(name="outp", bufs=4) as outp, \
             tc.tile_pool(name="ps_sc", bufs=2, space="PSUM") as ps_sc, \
             tc.tile_pool(name="ps_at", bufs=2, space="PSUM") as ps_at, \
             tc.tile_pool(name="ps_sm", bufs=2, space="PSUM") as ps_sm:
            def emit_oproj(c, ag):
                # partial o_proj for chunk c; emitted after the next chunk's
                # first attention heads so its ag-dependent matmuls never
                # starve the tensor queue at a chunk boundary
                for st in range(ST):
                    r0 = c * CH + st * P
                    for hp in range(HIDDEN // CH // 2):
                        pss = ps_sc.tile([P, 2, CH], f32, tag="sc", name="ops")
                        for dt in range(QH):
                            for hi in range(2):
                                h0 = hp * 2 + hi
                                nc.tensor.matmul(
                                    pss[:, hi, :],
                                    ag[:, dt, st * P:(st + 1) * P],
                                    wo_sb[:, dt, h0 * CH:(h0 + 1) * CH],
                                    start=(dt == 0),
                                    stop=(dt == QH - 1),
                                )
                        ob = outp.tile([P, 2, CH], bf16, tag="ob")
                        nc.vector.tensor_copy(out=ob, in_=pss)
                        nc.sync.dma_start(
                            out=out[r0:r0 + P, hp * 2 * CH:(hp * 2 + 2) * CH],
                            in_=ob,
                        )

            pending = None
            for c in range(NCH):
                cs = slice(c * CH, (c + 1) * CH)
                ntiles = (c + 1) * ST
                ag = agp.tile([P, QH, CH], bf16, tag="ag")
                npairs_off = c * ST // 2  # off-diagonal tile pairs per head
                for qh in range(QH):
                    at = ps_at.tile([P, CH], f32, tag="attn")
                    pr_all = prp.tile([P, ntiles, CH], bf16, tag="pr", name=f"pr{c}")
                    pr2 = prp.tile([P, max(npairs_off, 1), CH], bf16, tag="pr2",
                                   name=f"pr2{c}")
                    nquads = npairs_off // 2
                    pr4 = prp.tile([P, max(nquads, 1), CH], bf16, tag="pr4",
                                   name=f"pr4{c}")
                    # scores + exp (batched per tile-pair) + masked av
                    for tp in range((ntiles + 1) // 2):
                        npair = min(2, ntiles - 2 * tp)
                        sc2 = ps_sc.tile([P, 2, CH], f32, tag="sc")
                        for j in range(npair):
                            t = 2 * tp + j
                            o = t - c * ST  # >=0 on diagonal-chunk tiles
                            q0 = o * P if o > 0 else 0
                            nc.tensor.matmul(
                                sc2[:, j, q0:],
                                kro[:, t * P:(t + 1) * P],
                                qro[:, qh, c * CH + q0:(c + 1) * CH],
                                start=True,
                                stop=True,
                            )
                        # exp over both tiles in one ACTIVATE (full width;
                        # causally-dead columns are never read downstream)
                        nc.scalar.activation(
                            out=pr_all[:, 2 * tp:2 * tp + npair, :],
                            in_=sc2[:, 0:npair, :],
                            func=expf,
                            scale=SCALE,
                        )
                        for j in range(npair):
                            t = 2 * tp + j
                            o = t - c * ST
                            if o >= 0:
                                # triangular mask on the diagonal 128-col block
                                nc.vector.tensor_mul(
                                    pr_all[:, t, o * P:(o + 1) * P],
                                    pr_all[:, t, o * P:(o + 1) * P],
                                    msk_sb,
                                )
                        if tp < npairs_off:
                            # pre-sum off-diagonal pairs (then quads) for the
                            # denominator chain
                            nc.vector.tensor_add(
                                pr2[:, tp, :], pr_all[:, 2 * tp, :],
                                pr_all[:, 2 * tp + 1, :],
                            )
                            if tp % 2 == 1:
                                nc.vector.tensor_add(
                                    pr4[:, tp // 2, :], pr2[:, tp - 1, :],
                                    pr2[:, tp, :],
                                )
                        for j in range(npair):
                            t = 2 * tp + j
                            o = t - c * ST
                            q0 = o * P if o > 0 else 0
                            nc.tensor.matmul(
                                at[:, q0:],
                                v_sb[:, t, :],
                                pr_all[:, t, q0:],
                                start=(t == 0),
                                stop=(t == ntiles - 1),
                            )
                    # denominator, broadcast to all partitions by the all-2.0
                    # stationary operand (pairs off-diag, sliced singles on
                    # the diagonal chunk)
                    dn = ps_sm.tile([P, CH], f32, tag="sm", name="dn")
                    n_dn = nquads + ST
                    for i in range(n_dn):
                        if i < nquads:
                            rhs = pr4[:, i, :]
                        else:
                            o = i - nquads
                            q0 = o * P if o > 0 else 0
                            rhs = pr_all[:, c * ST + o, q0:]
                        nc.tensor.matmul(
                            dn[:, CH - rhs.shape[-1]:], twos, rhs,
                            start=(i == 0), stop=(i == n_dn - 1),
                        )
                    rc = workB.tile([P, CH], f32, tag="recip")
                    nc.vector.reciprocal_approx_fast(out=rc, in_=dn)
                    t3 = workB.tile([P, CH], f32, tag="t3")
                    # t3 = (tanh_gate + 1) * at ; with dn = 2*sum(exp) this
                    # yields ag = at * sigmoid_gate / sum(exp)
                    nc.vector.scalar_tensor_tensor(
                        out=t3, in0=gt[:, qh, cs], scalar=1.0, in1=at,
                        op0=mybir.AluOpType.add, op1=mybir.AluOpType.mult,
                    )
                    nc.vector.tensor_mul(ag[:, qh, :], t3, rc)

                    if qh == 1 and pending is not None:
                        emit_oproj(*pending)
                        pending = None
                pending = (c, ag)
            emit_oproj(*pending)

    nc.finalize()
    return nc


_PROGRAMS = {}


def _get_program(S=S_FULL):
    if S not in _PROGRAMS:
        _PROGRAMS[S] = build_program(S)
    return _PROGRAMS[S]


def _host_tables(position_ids_b, S):
    pos = np.asarray(position_ids_b, dtype=np.float32)  # [S]
    inv = 1.0 / (ROPE_THETA ** (np.arange(0, P, 2, dtype=np.float32) / P))  # [64]
    ang = np.concatenate([inv, inv]).astype(np.float32)[:, None] * pos[None, :]
    cosT = np.cos(ang).astype(BF16)
    sgn = np.where(np.arange(P) < 64, -1.0, 1.0).astype(np.float32)
    sinT = (np.sin(ang) * sgn[:, None]).astype(BF16)
    return cosT, sinT


def _causal_mask():
    r = np.arange(P)[:, None]
    j = np.arange(P)[None, :]
    return (r <= j).astype(BF16)


def make_in_maps(x, position_ids, Wq, Wk, Wv, Wo, Wg, bg, S=S_FULL):
    x = np.asarray(x, dtype=np.float32)
    msk = _causal_mask()
    maps = []
    xT_b = [np.ascontiguousarray(x[b, :S].T).astype(BF16) for b in range(B)]
    tabs = [_host_tables(np.asarray(position_ids)[b, :S], S) for b in range(B)]
    Wq = np.asarray(Wq, np.float32)
    Wk = np.asarray(Wk, np.float32)
    Wv = np.asarray(Wv, np.float32)
    Wo = np.asarray(Wo, np.float32)
    Wg = np.asarray(Wg, np.float32)
    bg = np.asarray(bg, np.float32)
    for core in range(8):
        b, g = core // 4, core % 4
        cosT, sinT = tabs[b]
        maps.append({
            "xT": xT_b[b],
            "wq": np.ascontiguousarray(Wq[:, g * DQ:(g + 1) * DQ]).astype(BF16),
            "wkv": np.ascontiguousarray(np.concatenate(
                [Wk[:, g * P:(g + 1) * P], Wv[:, g * P:(g + 1) * P]],
                axis=1)).astype(BF16),
            "wg": np.ascontiguousarray(Wg[:, g * DQ:(g + 1) * DQ]).astype(BF16),
            "wo": np.ascontiguousarray(Wo[g * DQ:(g + 1) * DQ, :]).astype(BF16),
            "bg": np.ascontiguousarray(0.5 * bg[g * DQ:(g + 1) * DQ]),
            "cosT": cosT,
            "sinT": sinT,
            "msk": msk,
        })
    return maps


def run(inputs, S=S_FULL, trace=False, **kw):
    nc = _get_program(S)
    maps = make_in_maps(S=S, **inputs)
    res = run_bass_kernel_spmd(nc, maps, core_ids=list(range(8)), trace=trace, **kw)
    out = np.zeros((B, S, HIDDEN), np.float32)
    for core in range(8):
        out[core // 4] += np.asarray(res.results[core]["out"], np.float32)
    return out, res


def kernel(x, position_ids, Wq, Wk, Wv, Wo, Wg, bg):
    out, _ = run(dict(x=x, position_ids=position_ids, Wq=Wq, Wk=Wk, Wv=Wv,
                      Wo=Wo, Wg=Wg, bg=bg))
    return out


# revision 33
# speedup vs baseline: 1.0152x; 1.0152x over previous
"""Trainium2 Bass kernel for LuluAttention (gated GQA attention + RoPE).

Sharding over 8 NeuronCores: core = b*4 + g where b = batch (2), g = head
group (4). Each core computes 4 Q heads + their shared KV head for one batch
element, plus the matching gate slice, and a partial o_proj output
(contraction over its 512 attn dims). Host sums the 4 partials per batch.

Two-pass structure per core:
  Pass A (chunks 0..3): x chunk load -> q/k projections + RoPE -> gate
    (sigmoid) -> v projection. All activations persist in SBUF in transposed
    layout ([dim, seq]) so attention needs no on-chip transposes.
  Pass B (chunks 0..3): causal attention (scoresT = kT.T @ qT per k-tile,
    exp batched 2 tiles per ACTIVATE, triangular-block mask on the diagonal
    128-col block only, attnT accumulated in PSUM), denominator via a dense
    ones-matmul chain over retained prob tiles, reciprocal_approx_fast,
    broadcast via K=1 matmul, gate+normalize muls, then partial o_proj.

This keeps the exp table set (pass B) and sigmoid set (pass A) from
thrashing, keeps TensorE dense (no long PE-idle gaps -> HAM stays at 8/8),
and slices diagonal-tile matmuls to skip the causally-masked column ranges.
"""

import numpy as np
import ml_dtypes
from contextlib import ExitStack

import concourse.bass as bass
import concourse.bacc as bacc
import concourse.tile as tile
from concourse import mybir
from concourse.bass_utils import run_bass_kernel_spmd

BF16 = ml_dtypes.bfloat16

HIDDEN = 2048
B = 2
S_FULL = 2048
P = 128
CH = 512               # seq chunk width
QH = 4                 # q heads per core
DQ = QH * P            # 512 q dims per core
KT = HIDDEN // P       # 16 contraction tiles
SCALE = 1.0 / float(np.sqrt(128.0))
ROPE_THETA = 10000.0


def build_program(S=S_FULL):
    f32 = mybir.dt.float32
    bf16 = mybir.dt.bfloat16
    tanh = mybir.ActivationFunctionType.Tanh
    expf = mybir.ActivationFunctionType.Exp

    NCH = S // CH
    ST = CH // P           # 4 seq sub-tiles per chunk

    nc = bacc.Bacc("TRN2", debug=False, target_bir_lowering=False)

    xT = nc.declare_dram_parameter("xT", [HIDDEN, S], bf16, False)
    wq = nc.declare_dram_parameter("wq", [HIDDEN, DQ], bf16, False)
    wkv = nc.declare_dram_parameter("wkv", [HIDDEN, 2 * P], bf16, False)
    wg = nc.declare_dram_parameter("wg", [HIDDEN, DQ], bf16, False)
    wo = nc.declare_dram_parameter("wo", [DQ, HIDDEN], bf16, False)
    bg = nc.declare_dram_parameter("bg", [DQ], f32, False)
    cosT = nc.declare_dram_parameter("cosT", [P, S], bf16, False)
    sinT = nc.declare_dram_parameter("sinT", [P, S], bf16, False)
    msk = nc.declare_dram_parameter("msk", [P, P], bf16, False)
    out = nc.declare_dram_parameter("out", [S, HIDDEN], bf16, True)

    with tile.TileContext(nc) as tc, ExitStack() as ctx:
        wpool = ctx.enter_context(tc.tile_pool(name="weights", bufs=1))
        qkv = ctx.enter_context(tc.tile_pool(name="qkv", bufs=1))

        # ---- persistent tiles; DMAs are issued in ramp-critical order ----
        # (sync-ring DMAs drain FIFO, so the first q-projection's operands
        # must be first in line: wq block 0, then the first x chunk.)
        # wq loaded in 4 contraction-row splits (1KB HBM rows, and the first
        # q chain can start as soon as split 0 lands via subtile deps)
        wq_sb = wpool.tile([P, KT, DQ], bf16, tag="wq")
        wkv_sb = wpool.tile([P, KT, 2 * P], bf16, tag="wkv")
        wk_sb = wkv_sb[:, :, 0:P]
        wv_sb = wkv_sb[:, :, P:2 * P]
        cos_sb = wpool.tile([P, S], bf16, tag="cos")
        sin_sb = wpool.tile([P, S], bf16, tag="sin")
        wg_sb = wpool.tile([P, KT, DQ], bf16, tag="wg")
        bg_sb = wpool.tile([P, QH], f32, tag="bg")
        msk_sb = wpool.tile([P, P], bf16, tag="msk")
        wo_sb = wpool.tile([P, QH, HIDDEN], bf16, tag="wo")

        def dma_wq_split(h):
            nc.sync.dma_start(
                out=wq_sb[:, h * KT // 4:(h + 1) * KT // 4, :],
                in_=wq[h * HIDDEN // 4:(h + 1) * HIDDEN // 4, :].rearrange(
                    "(kt p) n -> p kt n", p=P),
            )
        dma_wq_split(0)
        # denominator matmul: all-2.0 stationary operand broadcasts
        # 2*sum(exp) to every PSUM partition, and the 2x absorbs the gate's
        # (1 + tanh)/2 affine: ag = at*(1+tanh) * 1/(2*sum(exp))
        twos = wpool.tile([P, P], bf16, tag="twos")
        nc.vector.memset(twos, 2.0)

        # preload the exp/tanh ACT table set during the DMA ramp so no
        # table switch lands mid-kernel
        warm = wpool.tile([1, 2], f32, tag="warm")
        nc.vector.memset(warm, 1.0)
        nc.scalar.activation(out=warm[:, 1:2], in_=warm[:, 0:1], func=tanh)
        nc.scalar.activation(out=warm[:, 0:1], in_=warm[:, 1:2], func=expf)

        def load_weights_early():
            nc.sync.dma_start(out=cos_sb, in_=cosT[:, :])
            nc.sync.dma_start(out=sin_sb, in_=sinT[:, :])
            nc.sync.dma_start(
                out=wkv_sb, in_=wkv[:, :].rearrange("(kt p) n -> p kt n", p=P))
            nc.sync.dma_start(
                out=wg_sb, in_=wg[:, :].rearrange("(kt p) n -> p kt n", p=P))
            nc.sync.dma_start(out=bg_sb, in_=bg[:].rearrange("(h p) -> p h", p=P))
            nc.sync.dma_start(out=msk_sb, in_=msk[:, :])

        def load_weights_late():
            nc.sync.dma_start(
                out=wo_sb, in_=wo[:, :].rearrange("(dt p) n -> p dt n", p=P))

        # persistent per-core activations (transposed layouts)
        qro = qkv.tile([P, QH, S], bf16, tag="qro")
        kro = qkv.tile([P, S], bf16, tag="kro")
        v_sb = qkv.tile([P, S // P, P], bf16, tag="v")
        gt = qkv.tile([P, QH, S], bf16, tag="gt")

        # ================= PASS A: projections =================
        with tc.tile_pool(name="passA", bufs=3) as xpool, \
             tc.tile_pool(name="workA", bufs=6) as work, \
             tc.tile_pool(name="psA", bufs=6, space="PSUM") as psA:
            def load_xc(c):
                # split so the first q chain can start on the first piece;
                # for chunk 0, interleave with the wq row-splits so both
                # operands stream together
                xc = xpool.tile([P, KT, CH], bf16, tag="xc", name=f"xc{c}")
                ccs = slice(c * CH, (c + 1) * CH)
                nsp = 4 if c == 0 else 2
                for h in range(nsp):
                    kt0, kt1 = h * KT // nsp, (h + 1) * KT // nsp
                    nc.sync.dma_start(
                        out=xc[:, kt0:kt1, :],
                        in_=xT[kt0 * P:kt1 * P, ccs].rearrange(
                            "(kt p) n -> p kt n", p=P),
                    )
                    if c == 0 and h < 3:
                        dma_wq_split(h + 1)
                return xc

            xc_next = load_xc(0)
            load_weights_early()
            for c in range(NCH):
                cs = slice(c * CH, (c + 1) * CH)
                xc = xc_next
                if c + 1 < NCH:
                    xc_next = load_xc(c + 1)
                if c == 2:
                    load_weights_late()

                # q heads + k + v (transposed); RoPE applied to q/k out of PSUM
                for qh in range(QH + 2):
                    ps = psA.tile([P, CH], f32, tag="proj")
                    for kt in range(KT):
                        lhs = (
                            wq_sb[:, kt, qh * P:(qh + 1) * P]
                            if qh < QH
                            else (wk_sb if qh == QH else wv_sb)[:, kt, :]
                        )
                        nc.tensor.matmul(
                            ps, lhs, xc[:, kt, :], start=(kt == 0), stop=(kt == KT - 1)
                        )
                    if qh == QH + 1:
                        # v: cast out of PSUM, then xbar-transpose to [s, d]
                        # (scalar HWDGE ring: keeps it off the big-load ring)
                        vt = work.tile([P, CH], bf16, tag="vt")
                        nc.scalar.copy(out=vt, in_=ps)
                        for st in range(ST):
                            nc.scalar.dma_start_transpose(
                                out=v_sb[:, c * ST + st, :],
                                in_=vt[:, st * P:(st + 1) * P],
                            )
                        continue
                    qf = work.tile([P, CH], bf16, tag="qf")
                    nc.scalar.copy(out=qf, in_=ps)
                    # rotate-half via the scalar HWDGE ring: tiny and
                    # latency-critical, must not queue behind weight loads
                    rot = work.tile([P, CH], bf16, tag="rot")
                    nc.scalar.dma_start(out=rot[0:64, :], in_=qf[64:128, :])
                    nc.scalar.dma_start(out=rot[64:128, :], in_=qf[0:64, :])
                    t1 = work.tile([P, CH], bf16, tag="t1")
                    nc.vector.tensor_mul(t1, qf, cos_sb[:, cs])
                    t2 = work.tile([P, CH], bf16, tag="t2")
                    nc.vector.tensor_mul(t2, rot, sin_sb[:, cs])
                    dst = qro[:, qh, cs] if qh < QH else kro[:, cs]
                    nc.vector.tensor_add(dst, t1, t2)

                # gate heads: tanh((x @ Wg + bg)/2), transposed layout.
                # sigmoid = (1+tanh)/2 is finished in pass B's normalize
                # multiply (tanh shares the exp ACT table set; sigmoid won't).
                for qh in range(QH):
                    ps = psA.tile([P, CH], f32, tag="proj")
                    for kt in range(KT):
                        nc.tensor.matmul(
                            ps,
                            wg_sb[:, kt, qh * P:(qh + 1) * P],
                            xc[:, kt, :],
                            start=(kt == 0),
                            stop=(kt == KT - 1),
                        )
                    nc.scalar.activation(
                        out=gt[:, qh, cs],
                        in_=ps,
                        func=tanh,
                        bias=bg_sb[:, qh:qh + 1],
                        scale=0.5,
                    )

        # ================= PASS B: attention + o_proj =================
        # PSUM budget (8 banks): sc tag [P,2,CH] x2 bufs = 4 banks (shared by
        # attention score pairs and o_proj output pairs), attn tag x2 = 2,
        # sm tag x2 = 2 (denominator and broadcast cycle the same slots).
        with tc.tile_pool(name="prp", bufs=2) as prp, \
             tc.tile_pool(name="agp", bufs=2) as agp, \
             tc.tile_pool(name="workB", bufs=2) as workB, \
             tc.tile_pool(name="outp", bufs=4) as outp, \
             tc.tile_pool(name="ps_sc", bufs=2, space="PSUM") as ps_sc, \
             tc.tile_pool(name="ps_at", bufs=2, space="PSUM") as ps_at, \
             tc.tile_pool(name="ps_sm", bufs=2, space="PSUM") as ps_sm:
            def emit_oproj(c, ag):
                # partial o_proj for chunk c; emitted after the next chunk's
                # first attention heads so its ag-dependent matmuls never
                # starve the tensor queue at a chunk boundary
                for st in range(ST):
                    r0 = c * CH + st * P
                    for h0 in range(HIDDEN // CH):
                        # single-wide PSUM on the attn tag: the short CAST
                        # (658ns) stays ahead of the next 4-matmul chain, so
                        # slot WAR never stalls the tensor queue
                        pss = ps_at.tile([P, CH], f32, tag="attn", name="ops")
                        for dt in range(QH):
                            nc.tensor.matmul(
                                pss,
                                ag[:, dt, st * P:(st + 1) * P],
                                wo_sb[:, dt, h0 * CH:(h0 + 1) * CH],
                                start=(dt == 0),
                                stop=(dt == QH - 1),
                            )
                        ob = outp.tile([P, CH], bf16, tag="ob")
                        nc.vector.tensor_copy(out=ob, in_=pss)
                        nc.sync.dma_start(
                            out=out[r0:r0 + P, h0 * CH:(h0 + 1) * CH], in_=ob
                        )

            pending = None
            for c in range(NCH):
                cs = slice(c * CH, (c + 1) * CH)
                ntiles = (c + 1) * ST
                ag = agp.tile([P, QH, CH], bf16, tag="ag")
                npairs_off = c * ST // 2  # off-diagonal tile pairs per head
                for qh in range(QH):
                    at = ps_at.tile([P, CH], f32, tag="attn")
                    pr_all = prp.tile([P, ntiles, CH], bf16, tag="pr", name=f"pr{c}")
                    pr2 = prp.tile([P, max(npairs_off, 1), CH], bf16, tag="pr2",
                                   name=f"pr2{c}")
                    nquads = npairs_off // 2
                    pr4 = prp.tile([P, max(nquads, 1), CH], bf16, tag="pr4",
                                   name=f"pr4{c}")
                    # scores + exp (batched per tile-pair) + masked av
                    for tp in range((ntiles + 1) // 2):
                        npair = min(2, ntiles - 2 * tp)
                        sc2 = ps_sc.tile([P, 2, CH], f32, tag="sc")
                        for j in range(npair):
                            t = 2 * tp + j
                            o = t - c * ST  # >=0 on diagonal-chunk tiles
                            q0 = o * P if o > 0 else 0
                            nc.tensor.matmul(
                                sc2[:, j, q0:],
                                kro[:, t * P:(t + 1) * P],
                                qro[:, qh, c * CH + q0:(c + 1) * CH],
                                start=True,
                                stop=True,
                            )
                        # exp over both tiles in one ACTIVATE (full width;
                        # causally-dead columns are never read downstream)
                        nc.scalar.activation(
                            out=pr_all[:, 2 * tp:2 * tp + npair, :],
                            in_=sc2[:, 0:npair, :],
                            func=expf,
                            scale=SCALE,
                        )
                        for j in range(npair):
                            t = 2 * tp + j
                            o = t - c * ST
                            if o >= 0:
                                # triangular mask on the diagonal 128-col block
                                nc.vector.tensor_mul(
                                    pr_all[:, t, o * P:(o + 1) * P],
                                    pr_all[:, t, o * P:(o + 1) * P],
                                    msk_sb,
                                )
                        if tp < npairs_off:
                            # pre-sum off-diagonal pairs (then quads) for the
                            # denominator chain
                            nc.vector.tensor_add(
                                pr2[:, tp, :], pr_all[:, 2 * tp, :],
                                pr_all[:, 2 * tp + 1, :],
                            )
                            if tp % 2 == 1:
                                nc.vector.tensor_add(
                                    pr4[:, tp // 2, :], pr2[:, tp - 1, :],
                                    pr2[:, tp, :],
                                )
                        for j in range(npair):
                            t = 2 * tp + j
                            o = t - c * ST
                            q0 = o * P if o > 0 else 0
                            nc.tensor.matmul(
                                at[:, q0:],
                                v_sb[:, t, :],
                                pr_all[:, t, q0:],
                                start=(t == 0),
                                stop=(t == ntiles - 1),
                            )
                    # denominator, broadcast to all partitions by the all-2.0
                    # stationary operand (pairs off-diag, sliced singles on
                    # the diagonal chunk)
                    dn = ps_sm.tile([P, CH], f32, tag="sm", name="dn")
                    n_dn = nquads + ST
                    for i in range(n_dn):
                        if i < nquads:
                            rhs = pr4[:, i, :]
                        else:
                            o = i - nquads
                            q0 = o * P if o > 0 else 0
                            rhs = pr_all[:, c * ST + o, q0:]
                        nc.tensor.matmul(
                            dn[:, CH - rhs.shape[-1]:], twos, rhs,
                            start=(i == 0), stop=(i == n_dn - 1),
                        )
                    rc = workB.tile([P, CH], f32, tag="recip")
                    nc.vector.reciprocal_approx_fast(out=rc, in_=dn)
                    t3 = workB.tile([P, CH], f32, tag="t3")
                    # t3 = (tanh_gate + 1) * at ; with dn = 2*sum(exp) this
                    # yields ag = at * sigmoid_gate / sum(exp)
                    nc.vector.scalar_tensor_tensor(
                        out=t3, in0=gt[:, qh, cs], scalar=1.0, in1=at,
                        op0=mybir.AluOpType.add, op1=mybir.AluOpType.mult,
                    )
                    nc.vector.tensor_mul(ag[:, qh, :], t3, rc)

                    if qh == 1 and pending is not None:
                        emit_oproj(*pending)
                        pending = None
                pending = (c, ag)
            emit_oproj(*pending)

    nc.finalize()
    return nc


_PROGRAMS = {}


def _get_program(S=S_FULL):
    if S not in _PROGRAMS:
        _PROGRAMS[S] = build_program(S)
    return _PROGRAMS[S]


def _host_tables(position_ids_b, S):
    pos = np.asarray(position_ids_b, dtype=np.float32)  # [S]
    inv = 1.0 / (ROPE_THETA ** (np.arange(0, P, 2, dtype=np.float32) / P))  # [64]
    ang = np.concatenate([inv, inv]).astype(np.float32)[:, None] * pos[None, :]
    cosT = np.cos(ang).astype(BF16)
    sgn = np.where(np.arange(P) < 64, -1.0, 1.0).astype(np.float32)
    sinT = (np.sin(ang) * sgn[:, None]).astype(BF16)
    return cosT, sinT


def _causal_mask():
    r = np.arange(P)[:, None]
    j = np.arange(P)[None, :]
    return (r <= j).astype(BF16)


def make_in_maps(x, position_ids, Wq, Wk, Wv, Wo, Wg, bg, S=S_FULL):
    x = np.asarray(x, dtype=np.float32)
    msk = _causal_mask()
    maps = []
    xT_b = [np.ascontiguousarray(x[b, :S].T).astype(BF16) for b in range(B)]
    tabs = [_host_tables(np.asarray(position_ids)[b, :S], S) for b in range(B)]
    Wq = np.asarray(Wq, np.float32)
    Wk = np.asarray(Wk, np.float32)
    Wv = np.asarray(Wv, np.float32)
    Wo = np.asarray(Wo, np.float32)
    Wg = np.asarray(Wg, np.float32)
    bg = np.asarray(bg, np.float32)
    for core in range(8):
        b, g = core // 4, core % 4
        cosT, sinT = tabs[b]
        maps.append({
            "xT": xT_b[b],
            "wq": np.ascontiguousarray(Wq[:, g * DQ:(g + 1) * DQ]).astype(BF16),
            "wkv": np.ascontiguousarray(np.concatenate(
                [Wk[:, g * P:(g + 1) * P], Wv[:, g * P:(g + 1) * P]],
                axis=1)).astype(BF16),
            "wg": np.ascontiguousarray(Wg[:, g * DQ:(g + 1) * DQ]).astype(BF16),
            "wo": np.ascontiguousarray(Wo[g * DQ:(g + 1) * DQ, :]).astype(BF16),
            "bg": np.ascontiguousarray(0.5 * bg[g * DQ:(g + 1) * DQ]),
            "cosT": cosT,
            "sinT": sinT,
            "msk": msk,
        })
    return maps


def run(inputs, S=S_FULL, trace=False, **kw):
    nc = _get_program(S)
    maps = make_in_maps(S=S, **inputs)
    res = run_bass_kernel_spmd(nc, maps, core_ids=list(range(8)), trace=trace, **kw)
    out = np.zeros((B, S, HIDDEN), np.float32)
    for core in range(8):
        out[core // 4] += np.asarray(res.results[core]["out"], np.float32)
    return out, res


def kernel(x, position_ids, Wq, Wk, Wv, Wo, Wg, bg):
    out, _ = run(dict(x=x, position_ids=position_ids, Wq=Wq, Wk=Wk, Wv=Wv,
                      Wo=Wo, Wg=Wg, bg=bg))
    return out
